# revision 5
# baseline (speedup 1.0000x reference)
"""Multi-head attention (no qkv proj) + out_proj, sharded over 8 TRN2 cores.

Sharding: data-parallel over (batch, T-chunk): core i handles batch i//4,
query rows [ (i%4)*256, (i%4+1)*256 ).  Each core computes its full output
rows (all 16 heads + out_proj) independently -- no collectives needed.

Layout strategy ("T on the free dim" everywhere, zero on-device transposes):
  scoresT[s, t]  = K_h @ Q_h^T        lhsT = kT[64d, 128s]   rhs = qT[64d, 256t]
  rows[s, t]     = exp(scoresT*scale + biasT + mask)   (mask folded into biasT
                   on host; no max-subtract needed: |scores+bias| <= ~12)
  av[d(+1), t]   = [V_h | 1]^T @ rows  (65th row = softmax denominator)
  attnflatT[din, t] = av[0:64] * bcast(1/denom)
  outT[dout, t]  = W^T-chunks^T @ attnflatT + out_b  (bias per-partition, fused
                   into the PSUM->SBUF copy on the scalar engine)

Host pre-transposes q/k/w/bias so every DMA is contiguous.  Matmul operands
are declared float32r end-to-end (full-rate fp32 path, moving dim 256) --
walrus requires f32r consumers to see f32r-rounded producers.
"""

import numpy as np

import concourse.bass as bass
import concourse.mybir as mybir
import concourse.tile as tile
from concourse import bacc
from concourse.bass_utils import run_bass_kernel_spmd

F32 = mybir.dt.float32
F32R = mybir.dt.float32r

P = 128          # partitions
T = 256          # query rows per core
S = 1024         # key length
H = 16           # heads
HD = 64          # head dim
DM = 1024        # d_model
NS = S // P      # 8 s-chunks
ND = DM // P     # 8 d_model-chunks
SCALE = HD ** -0.5

AF = mybir.ActivationFunctionType
ALU = mybir.AluOpType


def build_bass():
    nc = bacc.Bacc()

    qT_d = nc.dram_tensor("qT", [DM, T], F32R, kind="ExternalInput")
    kT_d = nc.dram_tensor("kT", [DM, S], F32R, kind="ExternalInput")
    vaug_d = nc.dram_tensor("vaug", [S, H * (HD + 1)], F32R, kind="ExternalInput")
    biasT_d = nc.dram_tensor("biasT", [H, S, T], F32, kind="ExternalInput")
    wT_d = nc.dram_tensor("wT", [DM, DM], F32R, kind="ExternalInput")
    outb_d = nc.dram_tensor("outb", [P, ND], F32, kind="ExternalInput")
    outT_d = nc.dram_tensor("outT", [DM, T], F32, kind="ExternalOutput")

    with tile.TileContext(nc) as tc, nc.allow_low_precision(reason="float32r matmul pipeline"):
        with (
            tc.tile_pool(name="weights", bufs=1) as wpool,
            tc.tile_pool(name="bias", bufs=3) as bpool,
            tc.tile_pool(name="rows", bufs=3) as rpool,
            tc.tile_pool(name="small", bufs=2) as spool,
            tc.tile_pool(name="osb", bufs=1) as opool_sb,
        ):
            # ---- persistent weight tiles (issue critical-path DMAs first) ----
            qT_t = [wpool.tile([P, T], F32R, name=f"qT{c}", tag=f"qT{c}") for c in range(ND)]
            kT_t = [wpool.tile([P, S], F32R, name=f"kT{c}", tag=f"kT{c}") for c in range(ND)]
            vaug_t = [wpool.tile([P, H * (HD + 1)], F32R, name=f"va{c}", tag=f"va{c}") for c in range(NS)]
            wT_t = [wpool.tile([P, DM], F32R, name=f"wT{c}", tag=f"wT{c}") for c in range(ND)]
            outb_t = wpool.tile([P, ND], F32, name="outb", tag="outb")
            aflat_t = [wpool.tile([P, T], F32R, name=f"af{c}", tag=f"af{c}") for c in range(ND)]

            nc.sync.dma_start(out=qT_t[0][:], in_=qT_d[0:P, :])
            nc.sync.dma_start(out=kT_t[0][:], in_=kT_d[0:P, :])
            for c in range(NS):
                nc.sync.dma_start(out=vaug_t[c][:], in_=vaug_d[c * P:(c + 1) * P, :])
            for c in range(1, ND):
                nc.sync.dma_start(out=qT_t[c][:], in_=qT_d[c * P:(c + 1) * P, :])
                nc.sync.dma_start(out=kT_t[c][:], in_=kT_d[c * P:(c + 1) * P, :])
            nc.sync.dma_start(out=outb_t[:], in_=outb_d[:, :])

            with (
                tc.tile_pool(name="scps", bufs=4, space="PSUM") as scps,
                tc.tile_pool(name="avps", bufs=2, space="PSUM") as avps,
            ):
                for h in range(H):
                    c2, half = divmod(h, 2)
                    hp = slice(half * HD, (half + 1) * HD)

                    bias_sb = bpool.tile([P, NS * T], F32, name="bias", tag="bias")
                    nc.sync.dma_start(
                        out=bias_sb[:].rearrange("p (sc t) -> p sc t", t=T),
                        in_=biasT_d[h].rearrange("(sc p) t -> p sc t", p=P),
                    )

                    rows = rpool.tile([P, NS * T], F32R, name="rows", tag="rows")
                    for sc in range(NS):
                        sc_ps = scps.tile([P, T], F32, name="sc", tag="sc")
                        nc.tensor.matmul(
                            sc_ps[:],
                            (kT_t[c2][hp, sc * P:(sc + 1) * P]),
                            (qT_t[c2][hp, :]),
                            start=True, stop=True,
                        )
                        nc.vector.scalar_tensor_tensor(
                            out=rows[:, sc * T:(sc + 1) * T],
                            in0=sc_ps[:],
                            scalar=SCALE,
                            in1=bias_sb[:, sc * T:(sc + 1) * T],
                            op0=ALU.mult,
                            op1=ALU.add,
                        )
                    nc.scalar.activation(rows[:], rows[:], AF.Exp)

                    av_ps = avps.tile([HD + 1, T], F32, name="av", tag="av")
                    for sc in range(NS):
                        nc.tensor.matmul(
                            av_ps[:],
                            (vaug_t[sc][:, h * (HD + 1):(h + 1) * (HD + 1)]),
                            (rows[:, sc * T:(sc + 1) * T]),
                            start=(sc == 0), stop=(sc == NS - 1),
                        )

                    rcp = spool.tile([1, T], F32R, name="rcp", tag="rcp")
                    nc.vector.reciprocal(rcp[:], av_ps[HD:HD + 1, :])
                    bc_sb = spool.tile([HD, T], F32R, name="bc", tag="bc")
                    nc.gpsimd.partition_broadcast(bc_sb[:], rcp[:])
                    nc.vector.tensor_mul(
                        aflat_t[c2][hp, :], av_ps[0:HD, :], bc_sb[:],
                    )

            # ---- out_proj: outT[dout, t] = W^T @ attnflatT + out_b ----
            for c in range(ND):
                nc.sync.dma_start(out=wT_t[c][:], in_=wT_d[c * P:(c + 1) * P, :])

            osb = opool_sb.tile([P, ND * T], F32, name="osb", tag="osb")
            with tc.tile_pool(name="ops", bufs=4, space="PSUM") as ops:
                for dc in range(ND):
                    o_ps = ops.tile([P, T], F32, name="o", tag="o")
                    for dinc in range(ND):
                        nc.tensor.matmul(
                            o_ps[:],
                            (wT_t[dinc][:, dc * P:(dc + 1) * P]),
                            (aflat_t[dinc][:]),
                            start=(dinc == 0), stop=(dinc == ND - 1),
                        )
                    nc.scalar.activation(
                        osb[:, dc * T:(dc + 1) * T], o_ps[:], AF.Identity,
                        bias=outb_t[:, dc:dc + 1],
                    )

            nc.sync.dma_start(
                out=outT_d.rearrange("(dc p) t -> p dc t", p=P),
                in_=osb[:].rearrange("p (dc t) -> p dc t", t=T),
            )

    nc.finalize()
    return nc


_NC = None


def _get_nc():
    global _NC
    if _NC is None:
        _NC = build_bass()
    return _NC


def _make_in_maps(query, key, value, attn_bias, key_padding_mask, out_w, out_b):
    query = np.asarray(query, dtype=np.float32)
    key = np.asarray(key, dtype=np.float32)
    value = np.asarray(value, dtype=np.float32)
    attn_bias = np.asarray(attn_bias, dtype=np.float32)
    mask = np.asarray(key_padding_mask).astype(bool)
    out_w = np.asarray(out_w, dtype=np.float32)
    out_b = np.asarray(out_b, dtype=np.float32)

    wT = np.ascontiguousarray(out_w.T)
    outb = np.ascontiguousarray(out_b.reshape(ND, P).T)

    per_batch = {}
    for b in range(2):
        kT = np.ascontiguousarray(key[b].T)
        vaug = np.ones((S, H * (HD + 1)), np.float32)
        vaug.reshape(S, H, HD + 1)[:, :, :HD] = value[b].reshape(S, H, HD)
        per_batch[b] = (kT, vaug)

    in_maps = []
    for i in range(8):
        b, tc_i = divmod(i, 4)
        t0 = tc_i * T
        kT, vaug = per_batch[b]
        qT = np.ascontiguousarray(query[b, t0:t0 + T, :].T)
        biasT = np.ascontiguousarray(
            attn_bias[b, :, t0:t0 + T, :].transpose(0, 2, 1)
        )
        biasT[:, mask[b], :] -= 10000.0
        in_maps.append({
            "qT": qT, "kT": kT, "vaug": vaug, "biasT": biasT,
            "wT": wT, "outb": outb,
        })
    return in_maps


def run(inputs, trace=False, **run_kwargs):
    """Returns (output [2,1024,1024] f32, BassKernelResults)."""
    nc = _get_nc()
    in_maps = _make_in_maps(**inputs)
    res = run_bass_kernel_spmd(
        nc, in_maps, core_ids=list(range(8)), trace=trace, **run_kwargs
    )
    out = np.empty((2, S, DM), np.float32)
    for i, r in enumerate(res.results):
        b, tc_i = divmod(i, 4)
        out[b, tc_i * T:(tc_i + 1) * T, :] = r["outT"].T
    return out, res


def kernel(**inputs):
    out, _ = run(inputs, trace=False)
    return out


# revision 11
# speedup vs baseline: 1.2179x; 1.2179x over previous
"""Multi-head attention (no qkv proj) + out_proj, sharded over 8 TRN2 cores.

Sharding: data-parallel over (batch, T-chunk): core i handles batch i//4,
query rows [ (i%4)*256, (i%4+1)*256 ).  Each core computes its full output
rows (all 16 heads + out_proj) independently -- no collectives needed.

Layout strategy ("T on the free dim" everywhere, zero on-device transposes):
  scoresT[s, t]  = K_h @ Q_h^T        lhsT = kT[64d, 128s]   rhs = qT[64d, 256t]
  rows[s, t]     = scoresT*scale + biasT(+mask)   (mask folded into biasT on
                   host; no max-subtract needed: |scores+bias| <= ~10)
  exp[s, t]      = exp(rows - 2)  in fp16 (prescale keeps exp < 65504; the
                   constant cancels in the softmax normalization)
  av[d(+1), t]   = [V_h | 1]^T @ exp  (65th row = softmax denominator)
  attnflatT[din, t] = av[0:64] * bcast(1/denom)   (reciprocal batched over
                   4 heads; broadcast across partitions via GPSIMD)
  outT[dout, t]  = W^T-chunks^T @ attnflatT + out_b  (bias per-partition,
                   fused into the PSUM->SBUF copy on the scalar engine)

Host pre-transposes q/k/w/bias so every DMA is contiguous.  Matmul operands
are fp16 (full-rate PE path + fast weight load); accumulation is fp32 in
PSUM; softmax bias add runs in fp32.
"""

import numpy as np

import concourse.bass as bass
import concourse.mybir as mybir
import concourse.tile as tile
from concourse import bacc
from concourse.bass_utils import run_bass_kernel_spmd

F32 = mybir.dt.float32
F16 = mybir.dt.float16
BIAS_F16 = False  # attn_bias streamed as f32 (precision) vs f16 (half DMA)
BIAS_DT = F16 if BIAS_F16 else F32
BIAS_NP = np.float16 if BIAS_F16 else np.float32

P = 128          # partitions
T = 256          # query rows per core
S = 1024         # key length
H = 16           # heads
HD = 64          # head dim
DM = 1024        # d_model
NS = S // P      # 8 s-chunks
ND = DM // P     # 8 d_model-chunks
SCALE = HD ** -0.5
EXP_SHIFT = -2.0  # exp(x-2): keeps exp outputs < 65504 for fp16

AF = mybir.ActivationFunctionType
ALU = mybir.AluOpType


def build_bass():
    nc = bacc.Bacc()

    qT_d = nc.dram_tensor("qT", [DM, T], F16, kind="ExternalInput")
    kT_d = nc.dram_tensor("kT", [DM, S], F16, kind="ExternalInput")
    vaug_d = nc.dram_tensor("vaug", [S, H * (HD + 1)], F16, kind="ExternalInput")
    biasT_d = nc.dram_tensor("biasT", [H, S, T], BIAS_DT, kind="ExternalInput")
    wT_d = nc.dram_tensor("wT", [DM, DM], F16, kind="ExternalInput")
    outb_d = nc.dram_tensor("outb", [P, ND], F32, kind="ExternalInput")
    outT_d = nc.dram_tensor("outT", [DM, T], F32, kind="ExternalOutput")

    with tile.TileContext(nc) as tc, nc.allow_low_precision(reason="fp16 matmul pipeline"):
        with (
            tc.tile_pool(name="weights", bufs=1) as wpool,
            tc.tile_pool(name="bias", bufs=3) as bpool,
            tc.tile_pool(name="rows", bufs=3) as rpool,
            tc.tile_pool(name="small", bufs=2) as spool,
            tc.tile_pool(name="osb", bufs=1) as opool_sb,
        ):
            # ---- persistent weight tiles (issue critical-path DMAs first) ----
            qT_t = [wpool.tile([P, T], F16, name=f"qT{c}", tag=f"qT{c}") for c in range(ND)]
            kT_t = [wpool.tile([P, S], F16, name=f"kT{c}", tag=f"kT{c}") for c in range(ND)]
            vaug_t = [wpool.tile([P, H * (HD + 1)], F16, name=f"va{c}", tag=f"va{c}") for c in range(NS)]
            wT_t = [wpool.tile([P, DM], F16, name=f"wT{c}", tag=f"wT{c}") for c in range(ND)]
            outb_t = wpool.tile([P, ND], F32, name="outb", tag="outb")
            eshift_t = wpool.tile([P, 1], F32, name="eshift", tag="eshift")
            nc.vector.memset(eshift_t[:], EXP_SHIFT)
            ones_t = wpool.tile([P, HD], F16, name="ones", tag="ones")
            nc.vector.memset(ones_t[:], 1.0)
            aflat_t = [wpool.tile([P, T], F16, name=f"af{c}", tag=f"af{c}") for c in range(ND)]

            nc.sync.dma_start(out=qT_t[0][:], in_=qT_d[0:P, :])
            nc.sync.dma_start(out=kT_t[0][:], in_=kT_d[0:P, :])
            for c in range(NS):
                nc.sync.dma_start(out=vaug_t[c][:], in_=vaug_d[c * P:(c + 1) * P, :])
            for c in range(1, ND):
                nc.sync.dma_start(out=qT_t[c][:], in_=qT_d[c * P:(c + 1) * P, :])
                nc.sync.dma_start(out=kT_t[c][:], in_=kT_d[c * P:(c + 1) * P, :])
            nc.sync.dma_start(out=outb_t[:], in_=outb_d[:, :])

            with (
                tc.tile_pool(name="scps", bufs=2, space="PSUM") as scps,
                tc.tile_pool(name="avps", bufs=5, space="PSUM") as avps,
                tc.tile_pool(name="bcps", bufs=1, space="PSUM") as bcps,
            ):
                av_tiles = {}
                den4 = None
                for h in range(H):
                    c2, half = divmod(h, 2)
                    hp = slice(half * HD, (half + 1) * HD)
                    g4 = h % 4

                    bias_sb = bpool.tile([P, NS * T], BIAS_DT, name="bias", tag="bias")
                    nc.sync.dma_start(
                        out=bias_sb[:].rearrange("p (sc t) -> p sc t", t=T),
                        in_=biasT_d[h].rearrange("(sc p) t -> p sc t", p=P),
                    )

                    rows = rpool.tile([P, NS * T], F32, name="rows", tag="rows")
                    for sc2 in range(NS // 2):
                        sc_ps = scps.tile([P, 2 * T], F32, name="sc", tag="sc")
                        for j in range(2):
                            sc = 2 * sc2 + j
                            nc.tensor.matmul(
                                sc_ps[:, j * T:(j + 1) * T],
                                kT_t[c2][hp, sc * P:(sc + 1) * P],
                                qT_t[c2][hp, :],
                                start=True, stop=True,
                            )
                        nc.vector.scalar_tensor_tensor(
                            out=rows[:, sc2 * 2 * T:(sc2 + 1) * 2 * T],
                            in0=sc_ps[:],
                            scalar=SCALE,
                            in1=bias_sb[:, sc2 * 2 * T:(sc2 + 1) * 2 * T],
                            op0=ALU.mult,
                            op1=ALU.add,
                        )
                    expv = rpool.tile([P, NS * T], F16, name="expv", tag="expv")
                    nc.scalar.activation(expv[:], rows[:], AF.Exp, bias=eshift_t[:])

                    av_ps = avps.tile([HD + 1, T], F32, name="av", tag="av")
                    av_tiles[h] = av_ps
                    for sc in range(NS):
                        nc.tensor.matmul(
                            av_ps[:],
                            vaug_t[sc][:, h * (HD + 1):(h + 1) * (HD + 1)],
                            expv[:, sc * T:(sc + 1) * T],
                            start=(sc == 0), stop=(sc == NS - 1),
                        )
                    if g4 == 0:
                        den4 = spool.tile([P, T], F32, name="den4", tag="den4")
                        nc.vector.memset(den4[:], 1.0)
                    nc.scalar.copy(den4[32 * g4:32 * g4 + 1, :], av_ps[HD:HD + 1, :])

                    if g4 == 3:
                        rcp4 = spool.tile([P, T], F16, name="rcp4", tag="rcp4")
                        nc.vector.reciprocal(rcp4[:], den4[:])
                        rcpx = spool.tile([1, T], F16, name="rcpx", tag="rcpx")
                        nc.scalar.copy(rcpx[:], rcp4[96:97, :])
                        for hh in range(h - 3, h + 1):
                            cc2, hhalf = divmod(hh, 2)
                            hhp = slice(hhalf * HD, (hhalf + 1) * HD)
                            gg = 32 * (hh % 4)
                            rcp_row = rcpx[:] if gg == 96 else rcp4[gg:gg + 1, :]
                            bc_ps = bcps.tile([HD, T], F32, name="bcp", tag="bcp")
                            nc.tensor.matmul(
                                bc_ps[:], ones_t[0 if gg == 96 else gg:(0 if gg == 96 else gg) + 1, :], rcp_row,
                                start=True, stop=True,
                            )
                            bc_sb = spool.tile([HD, T], F32, name="bc", tag="bc", bufs=4)
                            nc.scalar.copy(bc_sb[:], bc_ps[:])
                            nc.vector.tensor_mul(
                                aflat_t[cc2][hhp, :], av_tiles[hh][0:HD, :], bc_sb[:],
                            )
                        av_tiles.clear()

            # ---- out_proj: outT[dout, t] = W^T @ attnflatT + out_b ----
            for c in range(ND):
                nc.sync.dma_start(out=wT_t[c][:], in_=wT_d[c * P:(c + 1) * P, :])

            osb = opool_sb.tile([P, ND * T], F32, name="osb", tag="osb")
            with tc.tile_pool(name="ops", bufs=4, space="PSUM") as ops:
                for dc in range(ND):
                    o_ps = ops.tile([P, T], F32, name="o", tag="o")
                    for dinc in range(ND):
                        nc.tensor.matmul(
                            o_ps[:],
                            wT_t[dinc][:, dc * P:(dc + 1) * P],
                            aflat_t[dinc][:],
                            start=(dinc == 0), stop=(dinc == ND - 1),
                        )
                    nc.scalar.activation(
                        osb[:, dc * T:(dc + 1) * T], o_ps[:], AF.Identity,
                        bias=outb_t[:, dc:dc + 1],
                    )

            nc.sync.dma_start(
                out=outT_d.rearrange("(dc p) t -> p dc t", p=P),
                in_=osb[:].rearrange("p (dc t) -> p dc t", t=T),
            )

    nc.finalize()
    return nc


_NC = None


def _get_nc():
    global _NC
    if _NC is None:
        _NC = build_bass()
    return _NC


def _make_in_maps(query, key, value, attn_bias, key_padding_mask, out_w, out_b):
    query = np.asarray(query, dtype=np.float32)
    key = np.asarray(key, dtype=np.float32)
    value = np.asarray(value, dtype=np.float32)
    attn_bias = np.asarray(attn_bias, dtype=np.float32)
    mask = np.asarray(key_padding_mask).astype(bool)
    out_w = np.asarray(out_w, dtype=np.float32)
    out_b = np.asarray(out_b, dtype=np.float32)

    wT = np.ascontiguousarray(out_w.T).astype(np.float16)
    outb = np.ascontiguousarray(out_b.reshape(ND, P).T)

    per_batch = {}
    for b in range(2):
        kT = np.ascontiguousarray(key[b].T).astype(np.float16)
        vaug = np.ones((S, H * (HD + 1)), np.float16)
        vaug.reshape(S, H, HD + 1)[:, :, :HD] = value[b].reshape(S, H, HD)
        per_batch[b] = (kT, vaug)

    in_maps = []
    for i in range(8):
        b, tc_i = divmod(i, 4)
        t0 = tc_i * T
        kT, vaug = per_batch[b]
        qT = np.ascontiguousarray(query[b, t0:t0 + T, :].T).astype(np.float16)
        biasT = np.ascontiguousarray(
            attn_bias[b, :, t0:t0 + T, :].transpose(0, 2, 1)
        )
        biasT[:, mask[b], :] -= 10000.0
        in_maps.append({
            "qT": qT, "kT": kT, "vaug": vaug, "biasT": biasT.astype(BIAS_NP),
            "wT": wT, "outb": outb,
        })
    return in_maps


def run(inputs, trace=False, **run_kwargs):
    """Returns (output [2,1024,1024] f32, BassKernelResults)."""
    nc = _get_nc()
    in_maps = _make_in_maps(**inputs)
    res = run_bass_kernel_spmd(
        nc, in_maps, core_ids=list(range(8)), trace=trace, **run_kwargs
    )
    out = np.empty((2, S, DM), np.float32)
    for i, r in enumerate(res.results):
        b, tc_i = divmod(i, 4)
        out[b, tc_i * T:(tc_i + 1) * T, :] = r["outT"].T
    return out, res


def kernel(**inputs):
    out, _ = run(inputs, trace=False)
    return out
